# revision 11
# baseline (speedup 1.0000x reference)
"""Bahdanau-attention Trainium2 kernel (8-core data-parallel over batch).

reference:
  f_proj = features @ W1 + b1              [B,T,H]
  h_proj = hidden @ W2 + b2                [B,1,H]
  score  = tanh(f_proj + h_proj) @ V + bV  [B,T,1]
  w      = softmax(mask(score))            [B,T,1]
  ctx    = sum_t w * features              [B,D]

Per-core layout (4 batches/core):
  - features shipped pre-transposed as featT [4, D, T] in fp32r
  - main matmul computes f_projT [h, t] tiles: lhsT = W1 d-tile (natural
    layout), rhs = featT d-tile, accumulated over d in PSUM; fp32r runs the
    PE at full rate (1 cycle/row for moving dim >= 256)
  - tanh fused on ScalarE with per-partition bias = h_proj + b1 + b2
  - score = V-column matmuls accumulated in a [1, 512] PSUM bank
  - masking/softmax on [1, T] rows; mask is exact in fp32 integer range
  - ctx = per-d-tile fused multiply+reduce (tensor_tensor_reduce) against a
    PE-broadcast of w, reading the SBUF-resident featT in place
"""
import os
from contextlib import ExitStack

import numpy as np

import concourse.bass as bass
import concourse.mybir as mybir
import concourse.tile as tile
from concourse import bacc
from concourse.bass_utils import run_bass_kernel_spmd
from neuron_dtypes._impl import fp32r as fp32r_impl

N_CORES = 8
B, T, D, H = 32, 2048, 1024, 1024
BL = B // N_CORES  # batches per core
NJ = D // 128      # d tiles
NH = H // 128      # h tiles
CH = 512           # t-chunk (one PSUM bank of fp32)
NCH = T // CH

F32 = mybir.dt.float32
F32R = mybir.dt.float32r
AF = mybir.ActivationFunctionType
OP = mybir.AluOpType

# Masking: s_masked = min(s, -4096*t + 4096*(L-1) + 1000).  All quantities
# are integers < 2^24 so fp32 arithmetic is exact: valid t gives a bound
# >= 1000 (never clips real scores), masked t gives <= -3096 (exp -> 0).
MASK_SCALE = 4096.0
MASK_MARGIN = 1000.0


def _round_fp32r(a: np.ndarray) -> np.ndarray:
    flat = np.ascontiguousarray(a, dtype=np.float32).ravel().view(np.uint32)
    out = fp32r_impl.cast_fp32_to_fp32r(len(flat), flat)
    return out.view(np.float32).reshape(a.shape)


def build_kernel(repeat: int = 1):
    nc = bacc.Bacc(None, target_bir_lowering=False)
    kin = dict(kind="ExternalInput")
    featT = nc.dram_tensor("featT", [BL, D, T], F32R, **kin)
    w1d = nc.dram_tensor("w1d", [D, H], F32R, **kin)
    w2d = nc.dram_tensor("w2d", [D, H], F32, **kin)
    vd = nc.dram_tensor("vd", [128, NH], F32R, **kin)      # V[j*128+p] -> [p, j]
    hidd = nc.dram_tensor("hidd", [128, NJ, BL], F32, **kin)
    b1d = nc.dram_tensor("b1d", [128, NH], F32, **kin)
    b2d = nc.dram_tensor("b2d", [128, NH], F32, **kin)
    bvd = nc.dram_tensor("bvd", [1, 1], F32, **kin)
    seqd = nc.dram_tensor("seqd", [1, BL], F32, **kin)
    niotad = nc.dram_tensor("niotad", [1, T], F32, **kin)  # -4096*t
    ctxd = nc.dram_tensor("ctxd", [BL, D], F32, kind="ExternalOutput")
    attnd = nc.dram_tensor("attnd", [BL, T], F32, kind="ExternalOutput")

    with tile.TileContext(nc) as tc, ExitStack() as stk:
        const = stk.enter_context(tc.tile_pool(name="const", bufs=1))
        w1p = stk.enter_context(tc.tile_pool(name="w1p", bufs=1))

        # ---- constants / small loads ----
        v_sb = const.tile([128, NH], F32R)
        nc.sync.dma_start(out=v_sb, in_=vd[:, :])
        hid_sb = const.tile([128, NJ, BL], F32)
        nc.sync.dma_start(out=hid_sb, in_=hidd[:, :, :])
        b1_sb = const.tile([128, NH], F32)
        nc.sync.dma_start(out=b1_sb, in_=b1d[:, :])
        b2_sb = const.tile([128, NH], F32)
        nc.sync.dma_start(out=b2_sb, in_=b2d[:, :])
        b12_sb = const.tile([128, NH], F32)
        nc.vector.tensor_add(b12_sb, b1_sb, b2_sb)
        bv_sb = const.tile([1, 1], F32)
        nc.sync.dma_start(out=bv_sb, in_=bvd[:, :])
        seq_sb = const.tile([1, BL], F32)
        nc.sync.dma_start(out=seq_sb, in_=seqd[:, :])
        # lb[b] = 4096*L - 3096 = 4096*(L-1) + 1000, exact in fp32
        lb_sb = const.tile([1, BL], F32)
        nc.vector.tensor_scalar(
            lb_sb, seq_sb, MASK_SCALE, MASK_MARGIN - MASK_SCALE, OP.mult, OP.add
        )
        niota_sb = const.tile([1, T], F32)
        nc.sync.dma_start(out=niota_sb, in_=niotad[:, :])
        ones_sb = const.tile([1, 128], F32)
        nc.vector.memset(ones_sb, 1.0)

        # ---- W1 resident in SBUF ----
        w1_sb = w1p.tile([128, NJ, H], F32R)
        for j in range(NJ):
            nc.sync.dma_start(out=w1_sb[:, j, :], in_=w1d[j * 128:(j + 1) * 128, :])

        # ---- preamble: bias[p, h, b] = (hidden @ W2)[h] + b1[h] + b2[h] ----
        bias_sb = const.tile([128, NH, BL], F32)
        with tc.tile_pool(name="w2p", bufs=3) as w2p, \
             tc.tile_pool(name="hpps", bufs=1, space="PSUM") as hpps:
            hp_ps = [hpps.tile([128, BL], F32, tag=f"hp{h}", name=f"hp{h}") for h in range(NH)]
            for j in range(NJ):
                w2t = w2p.tile([128, H], F32, tag="w2t")
                nc.sync.dma_start(out=w2t, in_=w2d[j * 128:(j + 1) * 128, :])
                for h in range(NH):
                    nc.tensor.matmul(
                        hp_ps[h],
                        w2t[:, h * 128:(h + 1) * 128],
                        hid_sb[:, j, :],
                        start=(j == 0),
                        stop=(j == NJ - 1),
                    )
            for h in range(NH):
                nc.vector.tensor_scalar(
                    bias_sb[:, h, :], hp_ps[h], b12_sb[:, h:h + 1], None, OP.add
                )

        # ---- main pipeline ----
        featp = stk.enter_context(tc.tile_pool(name="featp", bufs=15))
        gp = stk.enter_context(tc.tile_pool(name="gp", bufs=3))
        rowp = stk.enter_context(tc.tile_pool(name="rowp", bufs=2))
        smallp = stk.enter_context(tc.tile_pool(name="smallp", bufs=4))
        ctxp = stk.enter_context(tc.tile_pool(name="ctxp", bufs=2))
        scrp = stk.enter_context(tc.tile_pool(name="scrp", bufs=2))
        mps = stk.enter_context(tc.tile_pool(name="mps", bufs=3, space="PSUM"))
        sps = stk.enter_context(tc.tile_pool(name="sps", bufs=2, space="PSUM"))
        wps = stk.enter_context(tc.tile_pool(name="wps", bufs=2, space="PSUM"))

        for _rep in range(repeat):
            for b in range(BL):
                ft = []
                for j in range(NJ):
                    t_ = featp.tile([128, T], F32R, tag="ft", name=f"ft{b}_{j}")
                    nc.sync.dma_start(
                        out=t_, in_=featT[b, j * 128:(j + 1) * 128, :]
                    )
                    ft.append(t_)

                s_sb = rowp.tile([1, T], F32, tag="s")
                for c in range(NCH):
                    s_ps = sps.tile([1, CH], F32, tag="sps")
                    for h in range(NH):
                        ps = mps.tile([128, CH], F32, tag="mm")
                        for j in range(NJ):
                            nc.tensor.matmul(
                                ps,
                                w1_sb[:, j, h * 128:(h + 1) * 128],
                                ft[j][:, c * CH:(c + 1) * CH],
                                start=(j == 0),
                                stop=(j == NJ - 1),
                            )
                        g = gp.tile([128, CH], F32R, tag="g")
                        nc.scalar.activation(
                            out=g, in_=ps, func=AF.Tanh,
                            bias=bias_sb[:, h, b:b + 1], scale=1.0,
                        )
                        nc.tensor.matmul(
                            s_ps, v_sb[:, h:h + 1], g,
                            start=(h == 0), stop=(h == NH - 1),
                        )
                    # psum score row -> s_sb (+ bV)
                    nc.vector.tensor_scalar(
                        s_sb[0:1, c * CH:(c + 1) * CH], s_ps,
                        bv_sb[0:1, 0:1], None, OP.add,
                    )

                # mask: s = min(s, niota + lb[b])
                nc.vector.scalar_tensor_tensor(
                    out=s_sb, in0=niota_sb, scalar=lb_sb[0:1, b:b + 1],
                    in1=s_sb, op0=OP.add, op1=OP.min,
                )
                # softmax (1-lane row ops)
                nmx = smallp.tile([1, 1], F32, tag="nmx")
                nc.vector.tensor_reduce(
                    out=nmx, in_=s_sb, axis=mybir.AxisListType.X,
                    op=OP.max, negate=True,
                )
                zs = smallp.tile([1, 1], F32, tag="z")
                nc.scalar.activation(
                    out=s_sb, in_=s_sb, func=AF.Exp,
                    bias=nmx[0:1, 0:1], scale=1.0, accum_out=zs,
                )
                rz = smallp.tile([1, 1], F32, tag="rz")
                nc.vector.reciprocal(rz, zs)
                nc.vector.tensor_scalar(s_sb, s_sb, rz[0:1, 0:1], None, OP.mult)
                nc.sync.dma_start(out=attnd[b:b + 1, :], in_=s_sb)

                # ctx[d] = sum_t w[t] * featT[d, t]: fused multiply+sum per
                # (d-tile, chunk) into acc4 columns, folded at the end.
                ctx_sb = ctxp.tile([128, NJ], F32, tag="ctx")
                acc4 = ctxp.tile([128, NJ, NCH], F32, tag="acc4")
                for c in range(NCH):
                    wr_ps = wps.tile([128, CH], F32, tag="wr")
                    nc.tensor.matmul(
                        wr_ps, ones_sb, s_sb[0:1, c * CH:(c + 1) * CH],
                        start=True, stop=True,
                    )
                    for j in range(NJ):
                        fv = ft[j][:, c * CH:(c + 1) * CH].bitcast(F32)
                        scr = scrp.tile([128, CH], F32, tag="scr")
                        nc.vector.scalar_tensor_tensor(
                            out=scr, in0=fv, scalar=1.0, in1=wr_ps,
                            op0=OP.mult, op1=OP.mult,
                            accum_out=acc4[:, j, c:c + 1],
                        )
                for j in range(NJ):
                    nc.vector.tensor_reduce(
                        out=ctx_sb[:, j:j + 1], in_=acc4[:, j, :],
                        axis=mybir.AxisListType.X, op=OP.add,
                    )
                nc.sync.dma_start(
                    out=ctxd[b:b + 1, :].rearrange("o (j p) -> o p j", p=128),
                    in_=ctx_sb,
                )

    nc.finalize()
    return nc


_NC_CACHE: dict = {}


def _get_nc(repeat: int = 1):
    if repeat not in _NC_CACHE:
        _NC_CACHE[repeat] = build_kernel(repeat)
    return _NC_CACHE[repeat]


def make_in_maps(features, hidden, seq_len, W1, b1, W2, b2, V, bV):
    features = np.asarray(features, np.float32)
    hidden = np.asarray(hidden, np.float32)
    seq_len = np.asarray(seq_len)
    W1 = np.asarray(W1, np.float32)
    b1 = np.asarray(b1, np.float32)
    W2 = np.asarray(W2, np.float32)
    b2 = np.asarray(b2, np.float32)
    V = np.asarray(V, np.float32)
    bV = np.asarray(bV, np.float32)

    w1r = _round_fp32r(W1)
    vcol = _round_fp32r(V.reshape(NH, 128).T)          # [128, NH]
    b1c = np.ascontiguousarray(b1.reshape(NH, 128).T)  # [128, NH]
    b2c = np.ascontiguousarray(b2.reshape(NH, 128).T)
    bvv = bV.reshape(1, 1)
    niota = (-MASK_SCALE * np.arange(T, dtype=np.float32)).reshape(1, T)

    in_maps = []
    for c in range(N_CORES):
        sl = slice(BL * c, BL * (c + 1))
        ft = _round_fp32r(features[sl].transpose(0, 2, 1))  # [BL, D, T]
        hid = hidden[sl]  # [BL, D]
        hidt = np.ascontiguousarray(
            hid.T.reshape(NJ, 128, BL).transpose(1, 0, 2)
        )  # [128, NJ, BL]
        seqb = seq_len[sl].astype(np.float32).reshape(1, BL)
        in_maps.append({
            "featT": ft,
            "w1d": w1r,
            "w2d": W2,
            "vd": vcol,
            "hidd": hidt,
            "b1d": b1c,
            "b2d": b2c,
            "bvd": bvv,
            "seqd": seqb,
            "niotad": niota,
        })
    return in_maps


def kernel(features, hidden, seq_len, W1, b1, W2, b2, V, bV):
    nc = _get_nc()
    in_maps = make_in_maps(features, hidden, seq_len, W1, b1, W2, b2, V, bV)
    res = run_bass_kernel_spmd(nc, in_maps, list(range(N_CORES)))
    ctx = np.concatenate(
        [res.results[c]["ctxd"] for c in range(N_CORES)], axis=0
    ).astype(np.float32)
    attn = np.concatenate(
        [res.results[c]["attnd"] for c in range(N_CORES)], axis=0
    ).astype(np.float32)[:, :, None]
    return ctx, attn


# revision 12
# speedup vs baseline: 3.8635x; 3.8635x over previous
"""Bahdanau-attention Trainium2 kernel (8-core data-parallel over batch).

reference:
  f_proj = features @ W1 + b1              [B,T,H]
  h_proj = hidden @ W2 + b2                [B,1,H]
  score  = tanh(f_proj + h_proj) @ V + bV  [B,T,1]
  w      = softmax(mask(score))            [B,T,1]
  ctx    = sum_t w * features              [B,D]

Per-core layout (4 batches/core):
  - features shipped pre-transposed as featT [4, D, T] in fp32r
  - main matmul computes f_projT [h, t] tiles: lhsT = W1 d-tile (natural
    layout), rhs = featT d-tile, accumulated over d in PSUM; fp32r runs the
    PE at full rate (1 cycle/row for moving dim >= 256)
  - tanh fused on ScalarE with per-partition bias = h_proj + b1 + b2
  - score = V-column matmuls accumulated in a [1, 512] PSUM bank
  - masking/softmax on [1, T] rows; mask is exact in fp32 integer range
  - ctx = per-d-tile fused multiply+reduce (tensor_tensor_reduce) against a
    PE-broadcast of w, reading the SBUF-resident featT in place
"""
import os
from contextlib import ExitStack

import numpy as np

import concourse.bass as bass
import concourse.mybir as mybir
import concourse.tile as tile
from concourse import bacc
from concourse.bass_utils import run_bass_kernel_spmd
from neuron_dtypes._impl import fp32r as fp32r_impl

N_CORES = 8
B, T, D, H = 32, 2048, 1024, 1024
BL = B // N_CORES  # batches per core
NJ = D // 128      # d tiles
NH = H // 128      # h tiles
CH = 512           # t-chunk (one PSUM bank of fp32)
NCH = T // CH

F32 = mybir.dt.float32
F32R = mybir.dt.float32r
AF = mybir.ActivationFunctionType
OP = mybir.AluOpType

# Masking: s_masked = min(s, -4096*t + 4096*(L-1) + 1000).  All quantities
# are integers < 2^24 so fp32 arithmetic is exact: valid t gives a bound
# >= 1000 (never clips real scores), masked t gives <= -3096 (exp -> 0).
MASK_SCALE = 4096.0
MASK_MARGIN = 1000.0


def _round_fp32r(a: np.ndarray) -> np.ndarray:
    flat = np.ascontiguousarray(a, dtype=np.float32).ravel().view(np.uint32)
    out = fp32r_impl.cast_fp32_to_fp32r(len(flat), flat)
    return out.view(np.float32).reshape(a.shape)


def build_kernel(repeat: int = 1, loop_n: int = 0):
    nc = bacc.Bacc(None, target_bir_lowering=False)
    kin = dict(kind="ExternalInput")
    featT = nc.dram_tensor("featT", [BL, D, T], F32R, **kin)
    w1d = nc.dram_tensor("w1d", [D, H], F32R, **kin)
    w2d = nc.dram_tensor("w2d", [D, H], F32, **kin)
    vd = nc.dram_tensor("vd", [128, NH], F32R, **kin)      # V[j*128+p] -> [p, j]
    hidd = nc.dram_tensor("hidd", [128, NJ, BL], F32, **kin)
    b1d = nc.dram_tensor("b1d", [128, NH], F32, **kin)
    b2d = nc.dram_tensor("b2d", [128, NH], F32, **kin)
    bvd = nc.dram_tensor("bvd", [1, 1], F32, **kin)
    seqd = nc.dram_tensor("seqd", [1, BL], F32, **kin)
    niotad = nc.dram_tensor("niotad", [1, T], F32, **kin)  # -4096*t
    ctxd = nc.dram_tensor("ctxd", [BL, D], F32, kind="ExternalOutput")
    attnd = nc.dram_tensor("attnd", [BL, T], F32, kind="ExternalOutput")

    with tile.TileContext(nc) as tc, ExitStack() as stk:
        const = stk.enter_context(tc.tile_pool(name="const", bufs=1))
        w1p = stk.enter_context(tc.tile_pool(name="w1p", bufs=1))

        # ---- constants / small loads ----
        v_sb = const.tile([128, NH], F32R)
        nc.sync.dma_start(out=v_sb, in_=vd[:, :])
        hid_sb = const.tile([128, NJ, BL], F32)
        nc.sync.dma_start(out=hid_sb, in_=hidd[:, :, :])
        b1_sb = const.tile([128, NH], F32)
        nc.sync.dma_start(out=b1_sb, in_=b1d[:, :])
        b2_sb = const.tile([128, NH], F32)
        nc.sync.dma_start(out=b2_sb, in_=b2d[:, :])
        b12_sb = const.tile([128, NH], F32)
        nc.vector.tensor_add(b12_sb, b1_sb, b2_sb)
        bv_sb = const.tile([1, 1], F32)
        nc.sync.dma_start(out=bv_sb, in_=bvd[:, :])
        seq_sb = const.tile([1, BL], F32)
        nc.sync.dma_start(out=seq_sb, in_=seqd[:, :])
        # lb[b] = 4096*L - 3096 = 4096*(L-1) + 1000, exact in fp32
        lb_sb = const.tile([1, BL], F32)
        nc.vector.tensor_scalar(
            lb_sb, seq_sb, MASK_SCALE, MASK_MARGIN - MASK_SCALE, OP.mult, OP.add
        )
        niota_sb = const.tile([1, T], F32)
        nc.sync.dma_start(out=niota_sb, in_=niotad[:, :])
        ones_sb = const.tile([1, 128], F32)
        nc.vector.memset(ones_sb, 1.0)

        # ---- W1 resident in SBUF ----
        w1_sb = w1p.tile([128, NJ, H], F32R)
        for j in range(NJ):
            nc.sync.dma_start(out=w1_sb[:, j, :], in_=w1d[j * 128:(j + 1) * 128, :])

        # ---- preamble: bias[p, h, b] = (hidden @ W2)[h] + b1[h] + b2[h] ----
        bias_sb = const.tile([128, NH, BL], F32)
        with tc.tile_pool(name="w2p", bufs=3) as w2p, \
             tc.tile_pool(name="hpps", bufs=1, space="PSUM") as hpps:
            hp_ps = [hpps.tile([128, BL], F32, tag=f"hp{h}", name=f"hp{h}") for h in range(NH)]
            for j in range(NJ):
                w2t = w2p.tile([128, H], F32, tag="w2t")
                nc.sync.dma_start(out=w2t, in_=w2d[j * 128:(j + 1) * 128, :])
                for h in range(NH):
                    nc.tensor.matmul(
                        hp_ps[h],
                        w2t[:, h * 128:(h + 1) * 128],
                        hid_sb[:, j, :],
                        start=(j == 0),
                        stop=(j == NJ - 1),
                    )
            for h in range(NH):
                nc.vector.tensor_scalar(
                    bias_sb[:, h, :], hp_ps[h], b12_sb[:, h:h + 1], None, OP.add
                )

        # ---- main pipeline ----
        featp = stk.enter_context(tc.tile_pool(name="featp", bufs=15))
        gp = stk.enter_context(tc.tile_pool(name="gp", bufs=3))
        rowp = stk.enter_context(tc.tile_pool(name="rowp", bufs=2))
        smallp = stk.enter_context(tc.tile_pool(name="smallp", bufs=4))
        ctxp = stk.enter_context(tc.tile_pool(name="ctxp", bufs=2))
        scrp = stk.enter_context(tc.tile_pool(name="scrp", bufs=2))
        mps = stk.enter_context(tc.tile_pool(name="mps", bufs=3, space="PSUM"))
        sps = stk.enter_context(tc.tile_pool(name="sps", bufs=2, space="PSUM"))
        wps = stk.enter_context(tc.tile_pool(name="wps", bufs=2, space="PSUM"))

        def pipeline():
            for b in range(BL):
                ft = []
                for j in range(NJ):
                    t_ = featp.tile([128, T], F32R, tag="ft", name=f"ft{b}_{j}")
                    nc.sync.dma_start(
                        out=t_, in_=featT[b, j * 128:(j + 1) * 128, :]
                    )
                    ft.append(t_)

                s_sb = rowp.tile([1, T], F32, tag="s")
                for c in range(NCH):
                    s_ps = sps.tile([1, CH], F32, tag="sps")
                    for h in range(NH):
                        ps = mps.tile([128, CH], F32, tag="mm")
                        for j in range(NJ):
                            nc.tensor.matmul(
                                ps,
                                w1_sb[:, j, h * 128:(h + 1) * 128],
                                ft[j][:, c * CH:(c + 1) * CH],
                                start=(j == 0),
                                stop=(j == NJ - 1),
                            )
                        g = gp.tile([128, CH], F32R, tag="g")
                        nc.scalar.activation(
                            out=g, in_=ps, func=AF.Tanh,
                            bias=bias_sb[:, h, b:b + 1], scale=1.0,
                        )
                        nc.tensor.matmul(
                            s_ps, v_sb[:, h:h + 1], g,
                            start=(h == 0), stop=(h == NH - 1),
                        )
                    # psum score row -> s_sb (+ bV)
                    nc.vector.tensor_scalar(
                        s_sb[0:1, c * CH:(c + 1) * CH], s_ps,
                        bv_sb[0:1, 0:1], None, OP.add,
                    )

                # mask: s = min(s, niota + lb[b])
                nc.vector.scalar_tensor_tensor(
                    out=s_sb, in0=niota_sb, scalar=lb_sb[0:1, b:b + 1],
                    in1=s_sb, op0=OP.add, op1=OP.min,
                )
                # softmax (1-lane row ops)
                nmx = smallp.tile([1, 1], F32, tag="nmx")
                nc.vector.tensor_reduce(
                    out=nmx, in_=s_sb, axis=mybir.AxisListType.X,
                    op=OP.max, negate=True,
                )
                zs = smallp.tile([1, 1], F32, tag="z")
                nc.scalar.activation(
                    out=s_sb, in_=s_sb, func=AF.Exp,
                    bias=nmx[0:1, 0:1], scale=1.0, accum_out=zs,
                )
                rz = smallp.tile([1, 1], F32, tag="rz")
                nc.vector.reciprocal(rz, zs)
                nc.vector.tensor_scalar(s_sb, s_sb, rz[0:1, 0:1], None, OP.mult)
                nc.sync.dma_start(out=attnd[b:b + 1, :], in_=s_sb)

                # ctx[d] = sum_t w[t] * featT[d, t]: fused multiply+sum per
                # (d-tile, chunk) into acc4 columns, folded at the end.
                ctx_sb = ctxp.tile([128, NJ], F32, tag="ctx")
                acc4 = ctxp.tile([128, NJ, NCH], F32, tag="acc4")
                for c in range(NCH):
                    wr_ps = wps.tile([128, CH], F32, tag="wr")
                    nc.tensor.matmul(
                        wr_ps, ones_sb, s_sb[0:1, c * CH:(c + 1) * CH],
                        start=True, stop=True,
                    )
                    for j in range(NJ):
                        fv = ft[j][:, c * CH:(c + 1) * CH].bitcast(F32)
                        scr = scrp.tile([128, CH], F32, tag="scr")
                        nc.vector.scalar_tensor_tensor(
                            out=scr, in0=fv, scalar=1.0, in1=wr_ps,
                            op0=OP.mult, op1=OP.mult,
                            accum_out=acc4[:, j, c:c + 1],
                        )
                for j in range(NJ):
                    nc.vector.tensor_reduce(
                        out=ctx_sb[:, j:j + 1], in_=acc4[:, j, :],
                        axis=mybir.AxisListType.X, op=OP.add,
                    )
                nc.sync.dma_start(
                    out=ctxd[b:b + 1, :].rearrange("o (j p) -> o p j", p=128),
                    in_=ctx_sb,
                )

        if loop_n:
            with tc.For_i(0, loop_n, 1):
                pipeline()
        else:
            for _rep in range(repeat):
                pipeline()

    nc.finalize()
    return nc


_NC_CACHE: dict = {}


def _get_nc(repeat: int = 1):
    if repeat not in _NC_CACHE:
        _NC_CACHE[repeat] = build_kernel(repeat)
    return _NC_CACHE[repeat]


def make_in_maps(features, hidden, seq_len, W1, b1, W2, b2, V, bV):
    features = np.asarray(features, np.float32)
    hidden = np.asarray(hidden, np.float32)
    seq_len = np.asarray(seq_len)
    W1 = np.asarray(W1, np.float32)
    b1 = np.asarray(b1, np.float32)
    W2 = np.asarray(W2, np.float32)
    b2 = np.asarray(b2, np.float32)
    V = np.asarray(V, np.float32)
    bV = np.asarray(bV, np.float32)

    w1r = _round_fp32r(W1)
    vcol = _round_fp32r(V.reshape(NH, 128).T)          # [128, NH]
    b1c = np.ascontiguousarray(b1.reshape(NH, 128).T)  # [128, NH]
    b2c = np.ascontiguousarray(b2.reshape(NH, 128).T)
    bvv = bV.reshape(1, 1)
    niota = (-MASK_SCALE * np.arange(T, dtype=np.float32)).reshape(1, T)

    in_maps = []
    for c in range(N_CORES):
        sl = slice(BL * c, BL * (c + 1))
        ft = _round_fp32r(features[sl].transpose(0, 2, 1))  # [BL, D, T]
        hid = hidden[sl]  # [BL, D]
        hidt = np.ascontiguousarray(
            hid.T.reshape(NJ, 128, BL).transpose(1, 0, 2)
        )  # [128, NJ, BL]
        seqb = seq_len[sl].astype(np.float32).reshape(1, BL)
        in_maps.append({
            "featT": ft,
            "w1d": w1r,
            "w2d": W2,
            "vd": vcol,
            "hidd": hidt,
            "b1d": b1c,
            "b2d": b2c,
            "bvd": bvv,
            "seqd": seqb,
            "niotad": niota,
        })
    return in_maps


def kernel(features, hidden, seq_len, W1, b1, W2, b2, V, bV):
    nc = _get_nc()
    in_maps = make_in_maps(features, hidden, seq_len, W1, b1, W2, b2, V, bV)
    res = run_bass_kernel_spmd(nc, in_maps, list(range(N_CORES)))
    ctx = np.concatenate(
        [res.results[c]["ctxd"] for c in range(N_CORES)], axis=0
    ).astype(np.float32)
    attn = np.concatenate(
        [res.results[c]["attnd"] for c in range(N_CORES)], axis=0
    ).astype(np.float32)[:, :, None]
    return ctx, attn


# revision 17
# speedup vs baseline: 5.0601x; 1.3097x over previous
"""Bahdanau-attention Trainium2 kernel (8-core data-parallel over batch).

reference:
  f_proj = features @ W1 + b1              [B,T,H]
  h_proj = hidden @ W2 + b2                [B,1,H]
  score  = tanh(f_proj + h_proj) @ V + bV  [B,T,1]
  w      = softmax(mask(score))            [B,T,1]
  ctx    = sum_t w * features              [B,D]

Per-core layout (4 batches/core):
  - features shipped pre-transposed as featT [4, D, T] in fp32r, loaded in
    [128, 512] (d-tile, t-chunk) pieces ordered chunk-major so the first
    matmuls start as early as possible
  - main matmul computes f_projT [h, t] tiles: lhsT = W1 d-tile (natural
    layout), rhs = featT piece, accumulated over d in PSUM; fp32r runs the
    PE at full rate (1 cycle/row for moving dim >= 256)
  - tanh fused on ScalarE with per-partition bias = h_proj + b1 + b2
  - score = V-column matmuls accumulated in a [1, 512] PSUM bank; masking
    and partial maxima are done per chunk so only exp/normalize are serial
  - ctx = per-(d-tile, chunk) fused multiply+sum (scalar_tensor_tensor) on
    VectorE against a DMA partition-broadcast of w; the PE skips the whole
    ctx phase and rolls straight into the next batch's matmuls
"""
import os
from contextlib import ExitStack

import numpy as np

import concourse.bass as bass
import concourse.mybir as mybir
import concourse.tile as tile
from concourse import bacc
from concourse.bass_utils import run_bass_kernel_spmd
from neuron_dtypes._impl import fp32r as fp32r_impl

N_CORES = 8
B, T, D, H = 32, 2048, 1024, 1024
BL = B // N_CORES  # batches per core
NJ = D // 128      # d tiles
NH = H // 128      # h tiles
CH = 512           # t-chunk (one PSUM bank of fp32)
NCH = T // CH

F32 = mybir.dt.float32
F32R = mybir.dt.float32r
AF = mybir.ActivationFunctionType
OP = mybir.AluOpType

# Masking: s_masked = min(s, -4096*t + 4096*(L-1) + 1000).  All quantities
# are integers < 2^24 so fp32 arithmetic is exact: valid t gives a bound
# >= 1000 (never clips real scores), masked t gives <= -3096 (exp -> 0).
MASK_SCALE = 4096.0
MASK_MARGIN = 1000.0


def _round_fp32r(a: np.ndarray) -> np.ndarray:
    flat = np.ascontiguousarray(a, dtype=np.float32).ravel().view(np.uint32)
    out = fp32r_impl.cast_fp32_to_fp32r(len(flat), flat)
    return out.view(np.float32).reshape(a.shape)


def build_kernel(repeat: int = 1, loop_n: int = 0):
    nc = bacc.Bacc(None, target_bir_lowering=False)
    kin = dict(kind="ExternalInput")
    featT = nc.dram_tensor("featT", [BL, D, T], F32R, **kin)
    w1d = nc.dram_tensor("w1d", [D, H], F32R, **kin)
    w2td = nc.dram_tensor("w2td", [H, D], F32, **kin)
    vd = nc.dram_tensor("vd", [128, NH], F32R, **kin)      # V[j*128+p] -> [p, j]
    hidr = nc.dram_tensor("hidr", [BL, D], F32, **kin)
    b1d = nc.dram_tensor("b1d", [128, NH], F32, **kin)
    b2d = nc.dram_tensor("b2d", [128, NH], F32, **kin)
    bvd = nc.dram_tensor("bvd", [1, 1], F32, **kin)
    seqd = nc.dram_tensor("seqd", [1, BL], F32, **kin)
    niotad = nc.dram_tensor("niotad", [1, T], F32, **kin)  # -4096*t
    ctxd = nc.dram_tensor("ctxd", [BL, D], F32, kind="ExternalOutput")
    attnd = nc.dram_tensor("attnd", [BL, T], F32, kind="ExternalOutput")

    with tile.TileContext(nc) as tc, ExitStack() as stk:
        const = stk.enter_context(tc.tile_pool(name="const", bufs=1))
        w1p = stk.enter_context(tc.tile_pool(name="w1p", bufs=1))
        featp = stk.enter_context(tc.tile_pool(name="featp", bufs=56))
        gp = stk.enter_context(tc.tile_pool(name="gp", bufs=3))
        rowp = stk.enter_context(tc.tile_pool(name="rowp", bufs=2))
        smallp = stk.enter_context(tc.tile_pool(name="smallp", bufs=4))
        ctxp = stk.enter_context(tc.tile_pool(name="ctxp", bufs=2))
        scrp = stk.enter_context(tc.tile_pool(name="scrp", bufs=2))
        wrbp = stk.enter_context(tc.tile_pool(name="wrbp", bufs=4))

        # ---- constants / small loads ----
        v_sb = const.tile([128, NH], F32R)
        nc.sync.dma_start(out=v_sb, in_=vd[:, :])
        b1_sb = const.tile([128, NH], F32)
        nc.sync.dma_start(out=b1_sb, in_=b1d[:, :])
        b2_sb = const.tile([128, NH], F32)
        nc.sync.dma_start(out=b2_sb, in_=b2d[:, :])
        b12_sb = const.tile([128, NH], F32)
        nc.vector.tensor_add(b12_sb, b1_sb, b2_sb)
        bv_sb = const.tile([1, 1], F32)
        nc.sync.dma_start(out=bv_sb, in_=bvd[:, :])
        seq_sb = const.tile([1, BL], F32)
        nc.sync.dma_start(out=seq_sb, in_=seqd[:, :])
        # lb[b] = 4096*L - 3096 = 4096*(L-1) + 1000, exact in fp32
        lb_sb = const.tile([1, BL], F32)
        nc.vector.tensor_scalar(
            lb_sb, seq_sb, MASK_SCALE, MASK_MARGIN - MASK_SCALE, OP.mult, OP.add
        )
        niota_sb = const.tile([1, T], F32)
        nc.sync.dma_start(out=niota_sb, in_=niotad[:, :])

        def emit_feat(b, c, j, tag_sfx=""):
            t_ = featp.tile([128, CH], F32R, tag="ft", name=f"ft{tag_sfx}{b}_{j}_{c}")
            nc.sync.dma_start(
                out=t_,
                in_=featT[b, j * 128:(j + 1) * 128, c * CH:(c + 1) * CH],
            )
            return t_

        # ---- preamble: bias[p, h, b] = (hidden @ W2)[h] + b1[h] + b2[h] ----
        # Computed on VectorE (fused multiply+sum against a DMA-broadcast of
        # the hidden row) so the PE goes straight to the main matmuls.
        # W1 h-slices and batch-0 featT chunks interleave for fast startup.
        bias_sb = const.tile([128, NH, BL], F32)
        w1_sb = w1p.tile([128, NJ, H], F32R)
        b0_ft = {} if not loop_n else None
        NDH = 2  # d-halves for the hproj multiply
        with tc.tile_pool(name="w2tp", bufs=3) as w2tp, \
             tc.tile_pool(name="hbp", bufs=2) as hbp:
            hacc = const.tile([128, NH, BL, NDH], F32)
            hid_b = []
            for b in range(BL):
                hb = hbp.tile([128, D], F32, tag="hb", name=f"hb{b}")
                hsrc = hidr[b:b + 1, :]
                nc.sync.dma_start(out=hb, in_=bass.AP(
                    tensor=hsrc.tensor, offset=hsrc.offset,
                    ap=[[0, 128]] + list(hsrc.ap)[1:],
                ))
                hid_b.append(hb)
            for h in range(NH):
                w2t = w2tp.tile([128, D], F32, tag="w2t", name=f"w2t{h}")
                nc.sync.dma_start(out=w2t, in_=w2td[h * 128:(h + 1) * 128, :])
                for b in range(BL):
                    for k in range(NDH):
                        dsl = slice(k * (D // NDH), (k + 1) * (D // NDH))
                        scr = scrp.tile([128, CH], F32, tag="scr")
                        nc.vector.scalar_tensor_tensor(
                            out=scr, in0=w2t[:, dsl], scalar=1.0,
                            in1=hid_b[b][:, dsl], op0=OP.mult, op1=OP.mult,
                            accum_out=hacc[:, h, b, k:k + 1],
                        )
                # bias[:, h, :] = hacc[..0] + hacc[..1] + b12
                nc.vector.tensor_add(
                    bias_sb[:, h, :], hacc[:, h, :, 0], hacc[:, h, :, 1]
                )
                nc.vector.tensor_scalar(
                    bias_sb[:, h, :], bias_sb[:, h, :], b12_sb[:, h:h + 1],
                    None, OP.add,
                )
                # W1 h-slices (all j) right behind
                for j in range(NJ):
                    nc.sync.dma_start(
                        out=w1_sb[:, j, h * 128:(h + 1) * 128],
                        in_=w1d[j * 128:(j + 1) * 128, h * 128:(h + 1) * 128],
                    )
                # batch-0 featT chunks interleaved every other h
                if b0_ft is not None and h % 2 == 0 and h // 2 < NCH:
                    c = h // 2
                    for j in range(NJ):
                        b0_ft[(j, c)] = emit_feat(0, c, j, "p")

        mps = stk.enter_context(tc.tile_pool(name="mps", bufs=3, space="PSUM"))
        sps = stk.enter_context(tc.tile_pool(name="sps", bufs=2, space="PSUM"))

        def pipeline(first=False):
            for b in range(BL):
                if first and b == 0 and b0_ft is not None:
                    ft = b0_ft
                else:
                    ft = {}
                    for c in range(NCH):
                        for j in range(NJ):
                            ft[(j, c)] = emit_feat(b, c, j)

                s_sb = rowp.tile([1, T], F32, tag="s")
                pm = smallp.tile([1, NCH], F32, tag="pm")
                for c in range(NCH):
                    s_ps = sps.tile([1, CH], F32, tag="sps")
                    for h in range(NH):
                        ps = mps.tile([128, CH], F32, tag="mm")
                        for j in range(NJ):
                            nc.tensor.matmul(
                                ps,
                                w1_sb[:, j, h * 128:(h + 1) * 128],
                                ft[(j, c)],
                                start=(j == 0),
                                stop=(j == NJ - 1),
                            )
                        g = gp.tile([128, CH], F32R, tag="g")
                        nc.scalar.activation(
                            out=g, in_=ps, func=AF.Tanh,
                            bias=bias_sb[:, h, b:b + 1], scale=1.0,
                        )
                        nc.tensor.matmul(
                            s_ps, v_sb[:, h:h + 1], g,
                            start=(h == 0), stop=(h == NH - 1),
                        )
                    s_chunk = s_sb[0:1, c * CH:(c + 1) * CH]
                    # psum score row -> s_sb (+ bV)
                    nc.vector.tensor_scalar(
                        s_chunk, s_ps, bv_sb[0:1, 0:1], None, OP.add,
                    )
                    # mask chunk in place + partial max
                    nc.vector.scalar_tensor_tensor(
                        out=s_chunk, in0=niota_sb[0:1, c * CH:(c + 1) * CH],
                        scalar=lb_sb[0:1, b:b + 1], in1=s_chunk,
                        op0=OP.add, op1=OP.min,
                    )
                    nc.vector.tensor_reduce(
                        out=pm[0:1, c:c + 1], in_=s_chunk,
                        axis=mybir.AxisListType.X, op=OP.max,
                    )

                # softmax tail (short serial chain)
                nmx = smallp.tile([1, 1], F32, tag="nmx")
                nc.vector.tensor_reduce(
                    out=nmx, in_=pm, axis=mybir.AxisListType.X,
                    op=OP.max, negate=True,
                )
                zs = smallp.tile([1, 1], F32, tag="z")
                nc.scalar.activation(
                    out=s_sb, in_=s_sb, func=AF.Exp,
                    bias=nmx[0:1, 0:1], scale=1.0, accum_out=zs,
                )
                rz = smallp.tile([1, 1], F32, tag="rz")
                nc.vector.reciprocal(rz, zs)
                nc.vector.tensor_scalar(s_sb, s_sb, rz[0:1, 0:1], None, OP.mult)
                nc.sync.dma_start(out=attnd[b:b + 1, :], in_=s_sb)

                # ctx[d] = sum_t w[t] * featT[d, t]: DMA-broadcast each w
                # chunk across partitions, fused multiply+sum per (d-tile,
                # chunk) into acc4 columns, folded at the end.  PE-free.
                ctx_sb = ctxp.tile([128, NJ], F32, tag="ctx")
                acc4 = ctxp.tile([128, NJ, NCH], F32, tag="acc4")
                for c in range(NCH):
                    wsrc = attnd[b:b + 1, c * CH:(c + 1) * CH]
                    bcast = bass.AP(
                        tensor=wsrc.tensor, offset=wsrc.offset,
                        ap=[[0, 128]] + list(wsrc.ap)[1:],
                    )
                    wrb = wrbp.tile([128, CH], F32, tag="wrb")
                    nc.sync.dma_start(out=wrb, in_=bcast)
                    for j in range(NJ):
                        scr = scrp.tile([128, CH], F32, tag="scr")
                        nc.vector.scalar_tensor_tensor(
                            out=scr, in0=ft[(j, c)].bitcast(F32), scalar=1.0,
                            in1=wrb, op0=OP.mult, op1=OP.mult,
                            accum_out=acc4[:, j, c:c + 1],
                        )
                for j in range(NJ):
                    nc.vector.tensor_reduce(
                        out=ctx_sb[:, j:j + 1], in_=acc4[:, j, :],
                        axis=mybir.AxisListType.X, op=OP.add,
                    )
                nc.sync.dma_start(
                    out=ctxd[b:b + 1, :].rearrange("o (j p) -> o p j", p=128),
                    in_=ctx_sb,
                )

        if loop_n:
            with tc.For_i(0, loop_n, 1):
                pipeline()
        else:
            pipeline(first=True)
            for _rep in range(repeat - 1):
                pipeline()

    nc.finalize()
    return nc


_NC_CACHE: dict = {}


def _get_nc(repeat: int = 1):
    if repeat not in _NC_CACHE:
        _NC_CACHE[repeat] = build_kernel(repeat)
    return _NC_CACHE[repeat]


def make_in_maps(features, hidden, seq_len, W1, b1, W2, b2, V, bV):
    features = np.asarray(features, np.float32)
    hidden = np.asarray(hidden, np.float32)
    seq_len = np.asarray(seq_len)
    W1 = np.asarray(W1, np.float32)
    b1 = np.asarray(b1, np.float32)
    W2 = np.asarray(W2, np.float32)
    b2 = np.asarray(b2, np.float32)
    V = np.asarray(V, np.float32)
    bV = np.asarray(bV, np.float32)

    w1r = _round_fp32r(W1)
    w2t_host = np.ascontiguousarray(W2.T)  # [H, D]
    vcol = _round_fp32r(V.reshape(NH, 128).T)          # [128, NH]
    b1c = np.ascontiguousarray(b1.reshape(NH, 128).T)  # [128, NH]
    b2c = np.ascontiguousarray(b2.reshape(NH, 128).T)
    bvv = bV.reshape(1, 1)
    niota = (-MASK_SCALE * np.arange(T, dtype=np.float32)).reshape(1, T)

    in_maps = []
    for c in range(N_CORES):
        sl = slice(BL * c, BL * (c + 1))
        ft = _round_fp32r(features[sl].transpose(0, 2, 1))  # [BL, D, T]
        hid = np.ascontiguousarray(hidden[sl])  # [BL, D]
        seqb = seq_len[sl].astype(np.float32).reshape(1, BL)
        in_maps.append({
            "featT": ft,
            "w1d": w1r,
            "w2td": w2t_host,
            "vd": vcol,
            "hidr": hid,
            "b1d": b1c,
            "b2d": b2c,
            "bvd": bvv,
            "seqd": seqb,
            "niotad": niota,
        })
    return in_maps


def kernel(features, hidden, seq_len, W1, b1, W2, b2, V, bV):
    nc = _get_nc()
    in_maps = make_in_maps(features, hidden, seq_len, W1, b1, W2, b2, V, bV)
    res = run_bass_kernel_spmd(nc, in_maps, list(range(N_CORES)))
    ctx = np.concatenate(
        [res.results[c]["ctxd"] for c in range(N_CORES)], axis=0
    ).astype(np.float32)
    attn = np.concatenate(
        [res.results[c]["attnd"] for c in range(N_CORES)], axis=0
    ).astype(np.float32)[:, :, None]
    return ctx, attn
